# revision 28
# baseline (speedup 1.0000x reference)
"""Causal squeeze-excite 1d on 8 TRN2 NeuronCores.

Reference computation (per batch b):
    y = causal_ema(x)                      # y[t] = (1-a) y[t-1] + a x[t], y[0] = x[0]
    h = relu(w1 @ y[:, t] + b1)            # (32,)  per time step
    g = sigmoid(w2 @ h + b2)               # (512,) per time step
    out[:, t] = x[:, t] * g
Sharding: data-parallel over batch; core i gets x[2i:2i+2].

Structure (v4, fp16 IO):
  - x/out/weights travel as fp16: halves HBM traffic (the kernel is
    DMA-bound at ~358 GB/s/core); fp16's 2^-11 rounding is far inside
    tolerance.  Host lays DRAM out per (b, chunk) so every load is 128
    descriptors x one contiguous run (<= 8 KB).
  - EMA commutes with the channel projection: w1 @ ema(x) == ema((a*w1) @ x),
    so the DVE scan runs on a 32-row projected sequence, not [512, T].
  - Both batches stack in PSUM partitions (b0 rows 0-31, b1 rows 32-63 via
    PE tile placement), so ONE scan / ONE relu covers both batches.
    b0/b1 matmul chains are emission-interleaved so the two PE tiles
    co-execute (~2x PE throughput, robust to HAM K=4/8 throttling).
  - b1 rides the DVE relu (fused add+max); b2 rides the sigmoid
    ACTIVATE's per-partition bias.  ACT runs sigmoids only -- it is the
    busiest compute engine, so everything else is kept off it.
  - All loads issue up front on the Sync HWDGE ring; stores go out on
    the GpSimd SWDGE ring so a store trigger waiting on a sigmoid can
    never head-of-line block the load stream.  Chunk schedule
    1024/1024/1024/768/256 minimizes ACTIVATE instruction overhead
    while keeping the serial tail (last chunk) short.
"""

import numpy as np
from contextlib import ExitStack

import concourse.bass as bass
import concourse.bacc as bacc
import concourse.tile as tile
import concourse.mybir as mybir
from concourse.bass_utils import run_bass_kernel_spmd

F32 = mybir.dt.float32
F16 = mybir.dt.float16

N_CORES = 8
B, C, T = 16, 512, 4096
CSQ = 32          # squeeze dim
P = 128           # SBUF partitions
NCB = C // P      # channel blocks (4)
B_LOC = B // N_CORES          # batches per core (2)
M2 = B_LOC * CSQ  # stacked mm1 output rows (64)
TS = 512          # max matmul / scan sub-tile (one PSUM bank)
# Few, large chunks: each ACTIVATE (sigmoid) costs (N+352)/1.2 ns plus
# ~2 semaphore ops, so ACT time is minimized by the fewest chunks whose
# pg tiles still fit PSUM (pgp 2x3 banks + php 2x1 = 8 banks).
CHUNKS = [(0, 1024), (1024, 1536), (2560, 1536)]
NTH = len(CHUNKS)
TCMAX = max(c[1] for c in CHUNKS)


def _subtiles(tcc):
    """Split a chunk into <=TS sub-tiles."""
    out, o = [], 0
    while o < tcc:
        s = min(TS, tcc - o)
        out.append((o, s))
        o += s
    return out


def build_nc(B_loc, cw, C_=C, T_=T):
    assert B_loc == B_LOC
    d = 1.0 - 1.0 / cw
    assert sum(c[1] for c in CHUNKS) == T_

    nc = bacc.Bacc(trn_type="TRN2")
    # x/out DRAM layout: flat [p, b*T*NCB] with per-(b, chunk) contiguous
    # blocks laid [cb, t] (see make_in_maps).
    xin = nc.declare_dram_parameter("x", [P, B_loc * NCB * T_], F16,
                                    isOutput=False)
    w1e = nc.declare_dram_parameter("w1e", [P, NCB * CSQ], F16, isOutput=False)
    w2d = nc.declare_dram_parameter("w2d", [M2, C_], F16, isOutput=False)
    b1d = nc.declare_dram_parameter("b1d", [M2, 1], F32, isOutput=False)
    b2e = nc.declare_dram_parameter("b2e", [P, NCB], F32, isOutput=False)
    out = nc.declare_dram_parameter("out", [P, B_loc * NCB * T_], F16,
                                    isOutput=True)

    def dslice(dram, b, ci):
        t0, tcc = CHUNKS[ci]
        off = b * (NCB * T_) + t0 * NCB
        return dram[:, off:off + NCB * tcc]

    with ExitStack() as ctx:
        tc = ctx.enter_context(tile.TileContext(nc))
        const = ctx.enter_context(tc.tile_pool(name="const", bufs=1))
        # All chunks live in SBUF at once: loads all issue up front.
        xpool = ctx.enter_context(
            tc.tile_pool(name="xp", bufs=B_loc * NTH))
        opool = ctx.enter_context(tc.tile_pool(name="op", bufs=4))
        gpool = ctx.enter_context(tc.tile_pool(name="gp", bufs=4))
        upool = ctx.enter_context(tc.tile_pool(name="up", bufs=3))
        hpool = ctx.enter_context(tc.tile_pool(name="hp", bufs=3))
        cpool = ctx.enter_context(tc.tile_pool(name="cp", bufs=2))
        php = ctx.enter_context(tc.tile_pool(name="php", bufs=2, space="PSUM"))
        pgp = ctx.enter_context(tc.tile_pool(name="pgp", bufs=2, space="PSUM"))

        # Consts ride the Scalar HWDGE ring so the Sync ring starts on x
        # immediately (they finish long before the first sigmoid).
        w1_t = const.tile([P, NCB * CSQ], F16, tag="w1e")
        nc.scalar.dma_start(w1_t[:], w1e[:])
        w2_t = const.tile([M2, C_], F16, tag="w2d")
        nc.scalar.dma_start(w2_t[:], w2d[:])
        b1_t = const.tile([M2, 1], F32, tag="b1d")
        nc.scalar.dma_start(b1_t[:], b1d[:])
        b2_t = const.tile([P, NCB], F32, tag="b2e")
        nc.scalar.dma_start(b2_t[:], b2e[:])
        dconst = const.tile([M2, TS], F32, tag="dconst")
        nc.vector.memset(dconst[:], d)

        # Issue every load immediately; the Sync ring drains them at HBM
        # rate with nothing in the way.
        xts = {}
        xv3s = {}
        for ci in range(NTH):
            _, tcc = CHUNKS[ci]
            for b in range(B_loc):
                xts[(b, ci)] = xpool.tile([P, NCB * TCMAX], F16, tag="x",
                                          name=f"x{b}_{ci}")
                xv3s[(b, ci)] = dslice(xin, b, ci).rearrange(
                    "p (cb t) -> p cb t", cb=NCB, t=tcc)

        def xw3(b, ci):
            return xts[(b, ci)][:].rearrange("p (cb t) -> p cb t",
                                             cb=NCB, t=TCMAX)

        # Load order: chunk0's first sub-tile for BOTH batches leads (the
        # scan spine starts on it), then the rest of chunk0, then the
        # remaining chunks.  The Sync ring drains strictly FIFO.
        t00 = min(TS, CHUNKS[0][1])
        for b in range(B_loc):
            nc.sync.dma_start(xw3(b, 0)[:, :, 0:t00],
                              xv3s[(b, 0)][:, :, 0:t00])
        for b in range(B_loc):
            tcc = CHUNKS[0][1]
            if tcc > t00:
                nc.sync.dma_start(xw3(b, 0)[:, :, t00:tcc],
                                  xv3s[(b, 0)][:, :, t00:tcc])
        for ci in range(1, NTH):
            tcc = CHUNKS[ci][1]
            for b in range(B_loc):
                nc.sync.dma_start(xw3(b, ci)[:, :, 0:tcc], xv3s[(b, ci)])

        ph_pre = {}

        def phase1(ci):
            # mm1 for chunk ci, both batches stacked into one PSUM tile
            # (b0 -> rows 0-31, b1 -> rows 32-63): the two accumulation
            # chains are emission-interleaved so PE col-tiles (0,0) and
            # (0,32) co-execute.
            _, tcc = CHUNKS[ci]
            stl = _subtiles(tcc)
            xws_ = [xts[(b, ci)][:].rearrange("p (cb t) -> p cb t", cb=NCB,
                                              t=TCMAX)
                    for b in range(B_loc)]
            phs = []
            for o, s in stl:
                ph = php.tile([M2, TS], F32, tag="ph")
                for cb in range(NCB):
                    for b in range(B_loc):
                        nc.tensor.matmul(
                            ph[b * CSQ:(b + 1) * CSQ, 0:s],
                            w1_t[:, cb * CSQ:(cb + 1) * CSQ],
                            xws_[b][:, cb, o:o + s],
                            start=(cb == 0), stop=(cb == NCB - 1))
                phs.append(ph)
            ph_pre[ci] = phs

        carry = [None]
        hts = {}

        def phase2(th):
            # One scan per sub-tile + one fused relu per chunk, covering
            # BOTH batches (stacked rows).  Emitted BEFORE phase4(th-1)'s
            # muls so the DVE's in-order queue never parks the scan spine
            # behind ACT-paced gate multiplies.
            _, tcc = CHUNKS[th]
            stl = _subtiles(tcc)
            phs = ph_pre.pop(th)
            ut = upool.tile([M2, TCMAX], F32, tag="u")
            for k, (o, s) in enumerate(stl):
                if th == 0 and k == 0:
                    # u_0 = cw * p_0 makes y[0] = x[0] exact.
                    init = cpool.tile([M2, 1], F32, tag="c")
                    nc.vector.tensor_scalar_mul(
                        init[:], phs[k][:, 0:1], float(cw))
                    init_ap = init[:]
                else:
                    init_ap = carry[0]
                nc.vector.tensor_tensor_scan(
                    ut[:, o:o + s], dconst[:, 0:s],
                    phs[k][:, 0:s], init_ap,
                    mybir.AluOpType.mult, mybir.AluOpType.add)
                carry[0] = ut[:, o + s - 1:o + s]
            # Fused (u + b1) -> max(., 0) on the DVE keeps ACT free for
            # sigmoids.
            ht = hpool.tile([M2, TCMAX], F16, tag="h")
            nc.vector.tensor_scalar(
                ht[:, 0:tcc], ut[:, 0:tcc], b1_t[:], 0.0,
                mybir.AluOpType.add, mybir.AluOpType.max)
            hts[th] = ht

        phase1(0)
        phase2(0)
        for th in range(NTH):
            t0, tcc = CHUNKS[th]
            stl = _subtiles(tcc)
            ht = hts.pop(th)
            # Phase 3: mm2 + sigmoid per (b, cb); all sub-tiles of the
            # chunk land in one PSUM tile -> one sigmoid each, b2 riding
            # the ACTIVATE bias.  b0/b1 interleaved (PE row-tiles T0/T4).
            gts = [gpool.tile([P, NCB * TCMAX], F16, tag="g", name=f"g{b}")
                   for b in range(B_loc)]
            gws = [g[:].rearrange("p (cb t) -> p cb t", cb=NCB, t=TCMAX)
                   for g in gts]
            for cb in range(NCB):
                pgs = [pgp.tile([P, TCMAX], F32, tag="pg", name=f"pg{b}")
                       for b in range(B_loc)]
                for o, s in stl:
                    for b in range(B_loc):
                        nc.tensor.matmul(
                            pgs[b][:, o:o + s],
                            w2_t[b * CSQ:(b + 1) * CSQ, cb * P:(cb + 1) * P],
                            ht[b * CSQ:(b + 1) * CSQ, o:o + s],
                            start=True, stop=True)
                for b in range(B_loc):
                    nc.scalar.activation(
                        gws[b][:, cb, 0:tcc], pgs[b][:, 0:tcc],
                        mybir.ActivationFunctionType.Sigmoid,
                        bias=b2_t[:, cb:cb + 1])
            # Prefetch the NEXT chunk's mm1 (PE queue: right after this
            # chunk's mm2, so the PE chews mm1 while ACT sigmoids) and
            # its scans+relu (DVE queue: ahead of this chunk's muls).
            if th + 1 < NTH:
                phase1(th + 1)
                phase2(th + 1)
            # Phase 4: gate multiply + store per (batch, cb): each store
            # streams as soon as its cb's sigmoid lands, keeping the tail
            # (after the final sigmoid) to one small mul+store.  Stores
            # go out on the GpSimd SWDGE ring so they can never
            # head-of-line block the Sync ring's loads.
            for b in range(B_loc):
                ot = opool.tile([P, NCB * TCMAX], F16, tag="o", name=f"o{b}")
                ow = ot[:].rearrange("p (cb t) -> p cb t", cb=NCB, t=TCMAX)
                xw = xts.pop((b, th))[:].rearrange(
                    "p (cb t) -> p cb t", cb=NCB, t=TCMAX)
                dv = dslice(out, b, th).rearrange(
                    "p (cb t) -> p cb t", cb=NCB, t=tcc)
                for cb in range(NCB):
                    nc.vector.tensor_mul(
                        ow[:, cb, 0:tcc], xw[:, cb, 0:tcc],
                        gws[b][:, cb, 0:tcc])
                    nc.gpsimd.dma_start(dv[:, cb, :], ow[:, cb, 0:tcc])
    nc.compile()
    return nc


def make_in_maps(x, w1, b1, w2, b2, cw, n_cores=N_CORES):
    """Host-side shard + weight prep. Returns per-core input maps."""
    a = 1.0 / cw
    C_ = w2.shape[0]
    b_loc = x.shape[0] // n_cores

    w1sT = (np.asarray(w1) * a).T.astype(np.float32)      # [C, CSQ]
    w1e = np.empty((P, NCB * CSQ), dtype=np.float16)
    for cb in range(NCB):
        w1e[:, cb * CSQ:(cb + 1) * CSQ] = w1sT[cb * P:(cb + 1) * P, :]

    w2d = np.empty((M2, C_), dtype=np.float16)
    for b in range(b_loc):
        w2d[b * CSQ:(b + 1) * CSQ, :] = np.asarray(w2).T

    b1d = np.empty((M2, 1), dtype=np.float32)
    for b in range(b_loc):
        b1d[b * CSQ:(b + 1) * CSQ, 0] = np.asarray(b1)

    b2e = np.asarray(b2).astype(np.float32).reshape(NCB, P).T.copy()

    # [B, C, T] -> per-core flat [P, b*(chunk-major [cb, t])] fp16.
    x16 = np.asarray(x).astype(np.float16)
    x16 = x16.reshape(n_cores, b_loc, NCB, P, T)
    xf = np.empty((n_cores, P, b_loc * NCB * T), dtype=np.float16)
    for b in range(b_loc):
        for (t0, tcc) in CHUNKS:
            off = b * (NCB * T) + t0 * NCB
            blk = x16[:, b, :, :, t0:t0 + tcc]        # [core, cb, p, t]
            xf[:, :, off:off + NCB * tcc] = (
                blk.transpose(0, 2, 1, 3).reshape(n_cores, P, NCB * tcc))

    return [
        {"x": xf[i], "w1e": w1e, "w2d": w2d, "b1d": b1d, "b2e": b2e}
        for i in range(n_cores)
    ]


def unshard_out(results, n_cores=N_CORES, b_loc=B_LOC):
    """Per-core flat fp16 -> full [B, C, T] fp32."""
    o = np.stack([r["out"] for r in results], axis=0)  # [core, P, b*NCB*T]
    full = np.empty((n_cores, b_loc, NCB, P, T), dtype=np.float32)
    for b in range(b_loc):
        for (t0, tcc) in CHUNKS:
            off = b * (NCB * T) + t0 * NCB
            blk = o[:, :, off:off + NCB * tcc].reshape(n_cores, P, NCB, tcc)
            full[:, b, :, :, t0:t0 + tcc] = blk.transpose(0, 2, 1, 3)
    return full.reshape(B, C, T)


_NC_CACHE = {}


def kernel(x, w1, b1, w2, b2, context_window):
    cw = int(context_window)
    x = np.asarray(x)
    key = (cw, x.shape)
    if key not in _NC_CACHE:
        _NC_CACHE[key] = build_nc(x.shape[0] // N_CORES, cw)
    nc = _NC_CACHE[key]
    in_maps = make_in_maps(
        np.asarray(x), np.asarray(w1), np.asarray(b1),
        np.asarray(w2), np.asarray(b2), cw)
    res = run_bass_kernel_spmd(nc, in_maps, core_ids=list(range(N_CORES)))
    return unshard_out(res.results)


# revision 32
# speedup vs baseline: 1.0789x; 1.0789x over previous
"""Causal squeeze-excite 1d on 8 TRN2 NeuronCores.

Reference computation (per batch b):
    y = causal_ema(x)                      # y[t] = (1-a) y[t-1] + a x[t], y[0] = x[0]
    h = relu(w1 @ y[:, t] + b1)            # (32,)  per time step
    g = sigmoid(w2 @ h + b2)               # (512,) per time step
    out[:, t] = x[:, t] * g
Sharding: data-parallel over batch; core i gets x[2i:2i+2].

Structure (v4, fp16 IO):
  - x/out/weights travel as fp16: halves HBM traffic (the kernel is
    DMA-bound at ~358 GB/s/core); fp16's 2^-11 rounding is far inside
    tolerance.  Host lays DRAM out per (b, chunk) so every load is 128
    descriptors x one contiguous run (<= 8 KB).
  - EMA commutes with the channel projection: w1 @ ema(x) == ema((a*w1) @ x),
    so the DVE scan runs on a 32-row projected sequence, not [512, T].
  - Both batches stack in PSUM partitions (b0 rows 0-31, b1 rows 32-63 via
    PE tile placement), so ONE scan / ONE relu covers both batches.
    b0/b1 matmul chains are emission-interleaved so the two PE tiles
    co-execute (~2x PE throughput, robust to HAM K=4/8 throttling).
  - b1 rides the DVE relu (fused add+max); b2 rides the sigmoid
    ACTIVATE's per-partition bias.  ACT runs sigmoids only -- it is the
    busiest compute engine, so everything else is kept off it.
  - All loads issue up front on the Sync HWDGE ring; stores go out on
    the GpSimd SWDGE ring so a store trigger waiting on a sigmoid can
    never head-of-line block the load stream.  Chunk schedule
    1024/1024/1024/768/256 minimizes ACTIVATE instruction overhead
    while keeping the serial tail (last chunk) short.
"""

import numpy as np
from contextlib import ExitStack

import concourse.bass as bass
import concourse.bacc as bacc
import concourse.tile as tile
import concourse.mybir as mybir
from concourse.bass_utils import run_bass_kernel_spmd

F32 = mybir.dt.float32
F16 = mybir.dt.float16

N_CORES = 8
B, C, T = 16, 512, 4096
CSQ = 32          # squeeze dim
P = 128           # SBUF partitions
NCB = C // P      # channel blocks (4)
B_LOC = B // N_CORES          # batches per core (2)
M2 = B_LOC * CSQ  # stacked mm1 output rows (64)
TS = 512          # max matmul / scan sub-tile (one PSUM bank)
# Uniform 1024 chunks: each ACTIVATE (sigmoid) costs (N+352)/1.2 ns plus
# ~2 semaphore ops, so few chunks is good -- but pg tiles need pgp
# bufs=3 (2 banks each) so the ACT stream holds ~2.6us of buffered work
# while a next-chunk mm1 sub-tile is injected into the PE queue.
CHUNKS = [(0, 1024), (1024, 1024), (2048, 1024), (3072, 1024)]
NTH = len(CHUNKS)
TCMAX = max(c[1] for c in CHUNKS)


def _subtiles(tcc):
    """Split a chunk into <=TS sub-tiles."""
    out, o = [], 0
    while o < tcc:
        s = min(TS, tcc - o)
        out.append((o, s))
        o += s
    return out


def build_nc(B_loc, cw, C_=C, T_=T):
    assert B_loc == B_LOC
    d = 1.0 - 1.0 / cw
    assert sum(c[1] for c in CHUNKS) == T_

    nc = bacc.Bacc(trn_type="TRN2")
    # x/out DRAM layout: flat [p, b*T*NCB] with per-(b, chunk) contiguous
    # blocks laid [cb, t] (see make_in_maps).
    xin = nc.declare_dram_parameter("x", [P, B_loc * NCB * T_], F16,
                                    isOutput=False)
    w1e = nc.declare_dram_parameter("w1e", [P, NCB * CSQ], F16, isOutput=False)
    w2d = nc.declare_dram_parameter("w2d", [M2, C_], F16, isOutput=False)
    b1d = nc.declare_dram_parameter("b1d", [M2, 1], F32, isOutput=False)
    b2e = nc.declare_dram_parameter("b2e", [P, NCB], F32, isOutput=False)
    out = nc.declare_dram_parameter("out", [P, B_loc * NCB * T_], F16,
                                    isOutput=True)

    def dslice(dram, b, ci):
        t0, tcc = CHUNKS[ci]
        off = b * (NCB * T_) + t0 * NCB
        return dram[:, off:off + NCB * tcc]

    with ExitStack() as ctx:
        tc = ctx.enter_context(tile.TileContext(nc))
        const = ctx.enter_context(tc.tile_pool(name="const", bufs=1))
        # All chunks live in SBUF at once: loads all issue up front.
        xpool = ctx.enter_context(
            tc.tile_pool(name="xp", bufs=B_loc * NTH))
        opool = ctx.enter_context(tc.tile_pool(name="op", bufs=4))
        gpool = ctx.enter_context(tc.tile_pool(name="gp", bufs=4))
        upool = ctx.enter_context(tc.tile_pool(name="up", bufs=3))
        hpool = ctx.enter_context(tc.tile_pool(name="hp", bufs=3))
        cpool = ctx.enter_context(tc.tile_pool(name="cp", bufs=2))
        php = ctx.enter_context(tc.tile_pool(name="php", bufs=2, space="PSUM"))
        pgp = ctx.enter_context(tc.tile_pool(name="pgp", bufs=3, space="PSUM"))

        # Consts ride the Scalar HWDGE ring so the Sync ring starts on x
        # immediately (they finish long before the first sigmoid).
        w1_t = const.tile([P, NCB * CSQ], F16, tag="w1e")
        nc.scalar.dma_start(w1_t[:], w1e[:])
        w2_t = const.tile([M2, C_], F16, tag="w2d")
        nc.scalar.dma_start(w2_t[:], w2d[:])
        b1_t = const.tile([M2, 1], F32, tag="b1d")
        nc.scalar.dma_start(b1_t[:], b1d[:])
        b2_t = const.tile([P, NCB], F32, tag="b2e")
        nc.scalar.dma_start(b2_t[:], b2e[:])
        dconst = const.tile([M2, TS], F32, tag="dconst")
        nc.vector.memset(dconst[:], d)

        # Issue every load immediately; the Sync ring drains them at HBM
        # rate with nothing in the way.
        xts = {}
        xv3s = {}
        for ci in range(NTH):
            _, tcc = CHUNKS[ci]
            for b in range(B_loc):
                xts[(b, ci)] = xpool.tile([P, NCB * TCMAX], F16, tag="x",
                                          name=f"x{b}_{ci}")
                xv3s[(b, ci)] = dslice(xin, b, ci).rearrange(
                    "p (cb t) -> p cb t", cb=NCB, t=tcc)

        def xw3(b, ci):
            return xts[(b, ci)][:].rearrange("p (cb t) -> p cb t",
                                             cb=NCB, t=TCMAX)

        # Load order: chunk0's first sub-tile for BOTH batches leads (the
        # scan spine starts on it), then the rest of chunk0, then the
        # remaining chunks.  The Sync ring drains strictly FIFO.
        t00 = min(TS, CHUNKS[0][1])
        for b in range(B_loc):
            nc.sync.dma_start(xw3(b, 0)[:, :, 0:t00],
                              xv3s[(b, 0)][:, :, 0:t00])
        for b in range(B_loc):
            tcc = CHUNKS[0][1]
            if tcc > t00:
                nc.sync.dma_start(xw3(b, 0)[:, :, t00:tcc],
                                  xv3s[(b, 0)][:, :, t00:tcc])
        for ci in range(1, NTH):
            tcc = CHUNKS[ci][1]
            for b in range(B_loc):
                nc.sync.dma_start(xw3(b, ci)[:, :, 0:tcc], xv3s[(b, ci)])

        ph_pre = {}

        def phase1_piece(ci, k):
            # mm1 for sub-tile k of chunk ci, both batches stacked into
            # one PSUM tile (b0 -> rows 0-31, b1 -> rows 32-63): the two
            # accumulation chains are emission-interleaved so PE
            # col-tiles (0,0) and (0,32) co-execute.  Pieces are injected
            # between mm2 groups of the previous chunk so the in-order PE
            # queue never parks the next chunk's spine behind the
            # ACT-paced mm2 drain.
            o, s = _subtiles(CHUNKS[ci][1])[k]
            ph = php.tile([M2, TS], F32, tag="ph")
            for cb in range(NCB):
                for b in range(B_loc):
                    xw_ = xts[(b, ci)][:].rearrange(
                        "p (cb t) -> p cb t", cb=NCB, t=TCMAX)
                    nc.tensor.matmul(
                        ph[b * CSQ:(b + 1) * CSQ, 0:s],
                        w1_t[:, cb * CSQ:(cb + 1) * CSQ],
                        xw_[:, cb, o:o + s],
                        start=(cb == 0), stop=(cb == NCB - 1))
            ph_pre.setdefault(ci, []).append(ph)

        def phase1(ci):
            for k in range(len(_subtiles(CHUNKS[ci][1]))):
                phase1_piece(ci, k)

        carry = [None]
        hts = {}

        def phase2(th):
            # One scan per sub-tile + one fused relu per chunk, covering
            # BOTH batches (stacked rows).  Emitted BEFORE phase4(th-1)'s
            # muls so the DVE's in-order queue never parks the scan spine
            # behind ACT-paced gate multiplies.
            _, tcc = CHUNKS[th]
            stl = _subtiles(tcc)
            phs = ph_pre.pop(th)
            ut = upool.tile([M2, TCMAX], F32, tag="u")
            for k, (o, s) in enumerate(stl):
                if th == 0 and k == 0:
                    # u_0 = cw * p_0 makes y[0] = x[0] exact.
                    init = cpool.tile([M2, 1], F32, tag="c")
                    nc.vector.tensor_scalar_mul(
                        init[:], phs[k][:, 0:1], float(cw))
                    init_ap = init[:]
                else:
                    init_ap = carry[0]
                nc.vector.tensor_tensor_scan(
                    ut[:, o:o + s], dconst[:, 0:s],
                    phs[k][:, 0:s], init_ap,
                    mybir.AluOpType.mult, mybir.AluOpType.add)
                carry[0] = ut[:, o + s - 1:o + s]
            # Fused (u + b1) -> max(., 0) on the DVE keeps ACT free for
            # sigmoids.
            ht = hpool.tile([M2, TCMAX], F16, tag="h")
            nc.vector.tensor_scalar(
                ht[:, 0:tcc], ut[:, 0:tcc], b1_t[:], 0.0,
                mybir.AluOpType.add, mybir.AluOpType.max)
            hts[th] = ht

        phase1(0)
        phase2(0)
        for th in range(NTH):
            t0, tcc = CHUNKS[th]
            stl = _subtiles(tcc)
            ht = hts.pop(th)
            # Phase 3: mm2 + sigmoid per (b, cb); all sub-tiles of the
            # chunk land in one PSUM tile -> one sigmoid each, b2 riding
            # the ACTIVATE bias.  b0/b1 interleaved (PE row-tiles T0/T4).
            gts = [gpool.tile([P, NCB * TCMAX], F16, tag="g", name=f"g{b}")
                   for b in range(B_loc)]
            gws = [g[:].rearrange("p (cb t) -> p cb t", cb=NCB, t=TCMAX)
                   for g in gts]
            nsub = len(_subtiles(CHUNKS[th + 1][1])) if th + 1 < NTH else 0
            for cb in range(NCB):
                pgs = [pgp.tile([P, TCMAX], F32, tag="pg", name=f"pg{b}")
                       for b in range(B_loc)]
                for o, s in stl:
                    for b in range(B_loc):
                        nc.tensor.matmul(
                            pgs[b][:, o:o + s],
                            w2_t[b * CSQ:(b + 1) * CSQ, cb * P:(cb + 1) * P],
                            ht[b * CSQ:(b + 1) * CSQ, o:o + s],
                            start=True, stop=True)
                for b in range(B_loc):
                    nc.scalar.activation(
                        gws[b][:, cb, 0:tcc], pgs[b][:, 0:tcc],
                        mybir.ActivationFunctionType.Sigmoid,
                        bias=b2_t[:, cb:cb + 1])
                # Inject the next chunk's mm1 sub-tiles between mm2
                # groups (one piece per middle cb): each piece is ~8 MMs,
                # small enough that the 3-deep pg buffer keeps ACT fed.
                if 1 <= cb <= nsub:
                    phase1_piece(th + 1, cb - 1)
            # Next chunk's scans+relu go on the DVE queue AHEAD of this
            # chunk's ACT-paced gate multiplies.
            if th + 1 < NTH:
                phase2(th + 1)
            # Phase 4: gate multiply + store per (batch, cb): each store
            # streams as soon as its cb's sigmoid lands, keeping the tail
            # (after the final sigmoid) to one small mul+store.  Stores
            # go out on the GpSimd SWDGE ring so they can never
            # head-of-line block the Sync ring's loads.
            for b in range(B_loc):
                ot = opool.tile([P, NCB * TCMAX], F16, tag="o", name=f"o{b}")
                ow = ot[:].rearrange("p (cb t) -> p cb t", cb=NCB, t=TCMAX)
                xw = xts.pop((b, th))[:].rearrange(
                    "p (cb t) -> p cb t", cb=NCB, t=TCMAX)
                dv = dslice(out, b, th).rearrange(
                    "p (cb t) -> p cb t", cb=NCB, t=tcc)
                for cb in range(NCB):
                    nc.vector.tensor_mul(
                        ow[:, cb, 0:tcc], xw[:, cb, 0:tcc],
                        gws[b][:, cb, 0:tcc])
                    nc.gpsimd.dma_start(dv[:, cb, :], ow[:, cb, 0:tcc])
    nc.compile()
    return nc


def make_in_maps(x, w1, b1, w2, b2, cw, n_cores=N_CORES):
    """Host-side shard + weight prep. Returns per-core input maps."""
    a = 1.0 / cw
    C_ = w2.shape[0]
    b_loc = x.shape[0] // n_cores

    w1sT = (np.asarray(w1) * a).T.astype(np.float32)      # [C, CSQ]
    w1e = np.empty((P, NCB * CSQ), dtype=np.float16)
    for cb in range(NCB):
        w1e[:, cb * CSQ:(cb + 1) * CSQ] = w1sT[cb * P:(cb + 1) * P, :]

    w2d = np.empty((M2, C_), dtype=np.float16)
    for b in range(b_loc):
        w2d[b * CSQ:(b + 1) * CSQ, :] = np.asarray(w2).T

    b1d = np.empty((M2, 1), dtype=np.float32)
    for b in range(b_loc):
        b1d[b * CSQ:(b + 1) * CSQ, 0] = np.asarray(b1)

    b2e = np.asarray(b2).astype(np.float32).reshape(NCB, P).T.copy()

    # [B, C, T] -> per-core flat [P, b*(chunk-major [cb, t])] fp16.
    x16 = np.asarray(x).astype(np.float16)
    x16 = x16.reshape(n_cores, b_loc, NCB, P, T)
    xf = np.empty((n_cores, P, b_loc * NCB * T), dtype=np.float16)
    for b in range(b_loc):
        for (t0, tcc) in CHUNKS:
            off = b * (NCB * T) + t0 * NCB
            blk = x16[:, b, :, :, t0:t0 + tcc]        # [core, cb, p, t]
            xf[:, :, off:off + NCB * tcc] = (
                blk.transpose(0, 2, 1, 3).reshape(n_cores, P, NCB * tcc))

    return [
        {"x": xf[i], "w1e": w1e, "w2d": w2d, "b1d": b1d, "b2e": b2e}
        for i in range(n_cores)
    ]


def unshard_out(results, n_cores=N_CORES, b_loc=B_LOC):
    """Per-core flat fp16 -> full [B, C, T] fp32."""
    o = np.stack([r["out"] for r in results], axis=0)  # [core, P, b*NCB*T]
    full = np.empty((n_cores, b_loc, NCB, P, T), dtype=np.float32)
    for b in range(b_loc):
        for (t0, tcc) in CHUNKS:
            off = b * (NCB * T) + t0 * NCB
            blk = o[:, :, off:off + NCB * tcc].reshape(n_cores, P, NCB, tcc)
            full[:, b, :, :, t0:t0 + tcc] = blk.transpose(0, 2, 1, 3)
    return full.reshape(B, C, T)


_NC_CACHE = {}


def kernel(x, w1, b1, w2, b2, context_window):
    cw = int(context_window)
    x = np.asarray(x)
    key = (cw, x.shape)
    if key not in _NC_CACHE:
        _NC_CACHE[key] = build_nc(x.shape[0] // N_CORES, cw)
    nc = _NC_CACHE[key]
    in_maps = make_in_maps(
        np.asarray(x), np.asarray(w1), np.asarray(b1),
        np.asarray(w2), np.asarray(b2), cw)
    res = run_bass_kernel_spmd(nc, in_maps, core_ids=list(range(N_CORES)))
    return unshard_out(res.results)


# revision 33
# speedup vs baseline: 1.1451x; 1.0614x over previous
"""Causal squeeze-excite 1d on 8 TRN2 NeuronCores.

Reference computation (per batch b):
    y = causal_ema(x)                      # y[t] = (1-a) y[t-1] + a x[t], y[0] = x[0]
    h = relu(w1 @ y[:, t] + b1)            # (32,)  per time step
    g = sigmoid(w2 @ h + b2)               # (512,) per time step
    out[:, t] = x[:, t] * g
Sharding: data-parallel over batch; core i gets x[2i:2i+2].

Structure (fp16 IO):
  - x/out/weights travel as fp16: halves HBM traffic (the kernel is
    DMA-bound at ~358 GB/s/core); fp16's 2^-11 rounding is far inside
    tolerance.  Host lays DRAM out as [128p, b, chunk, cb, t] so every
    load is 128 descriptors x 8 KB contiguous.
  - EMA commutes with the channel projection: w1 @ ema(x) == ema((a*w1) @ x),
    so the DVE scan runs on a 32-row projected sequence, not [512, T].
  - Both batches stack in PSUM partitions (b0 rows 0-31, b1 rows 32-63 via
    PE tile placement), so ONE scan / ONE relu instruction covers both
    batches.  b0/b1 matmul chains are emission-interleaved so the two PE
    tiles co-execute (~2x PE throughput, robust to HAM throttling).
  - b1 rides the DVE relu (fused add+max); b2 rides the sigmoid
    ACTIVATE's per-partition bias.  ACT runs sigmoids only -- it is the
    busiest compute engine, so everything else stays off it; uniform
    1024-col chunks minimize the (N+352)-cycle ACTIVATE overhead.
"""

import numpy as np
from contextlib import ExitStack

import concourse.bass as bass
import concourse.bacc as bacc
import concourse.tile as tile
import concourse.mybir as mybir
from concourse.bass_utils import run_bass_kernel_spmd

F32 = mybir.dt.float32
F16 = mybir.dt.float16

N_CORES = 8
B, C, T = 16, 512, 4096
CSQ = 32          # squeeze dim
P = 128           # SBUF partitions
NCB = C // P      # channel blocks (4)
B_LOC = B // N_CORES          # batches per core (2)
M2 = B_LOC * CSQ  # stacked mm1 output rows (64)
Tc = 1024         # time chunk
NCI = T // Tc     # DRAM chunk blocks (4)
TS = 512          # matmul / scan sub-tile (one PSUM bank)
PREF = 2          # load prefetch distance, in chunks
CHUNKS = [(0, 1024), (1024, 1024), (2048, 1024), (3072, 1024)]
NTH = len(CHUNKS)


def build_nc(B_loc, cw, C_=C, T_=T):
    assert B_loc == B_LOC
    d = 1.0 - 1.0 / cw

    nc = bacc.Bacc(trn_type="TRN2")
    # x/out DRAM layout: [p, b, ci, cb, t] (fp16).  One load per (b, ci)
    # is 128 x 8KB contiguous; one store per (b, ci, cb-pair) is 128 x 4KB.
    xin = nc.declare_dram_parameter("x", [P, B_loc * NCI * NCB * Tc], F16,
                                    isOutput=False)
    w1e = nc.declare_dram_parameter("w1e", [P, NCB * CSQ], F16, isOutput=False)
    w2d = nc.declare_dram_parameter("w2d", [M2, C_], F16, isOutput=False)
    b1d = nc.declare_dram_parameter("b1d", [M2, 1], F32, isOutput=False)
    b2e = nc.declare_dram_parameter("b2e", [P, NCB], F32, isOutput=False)
    out = nc.declare_dram_parameter("out", [P, B_loc * NCI * NCB * Tc], F16,
                                    isOutput=True)

    xv = xin.rearrange("p (b ci cb t) -> p b ci cb t", b=B_loc, ci=NCI, cb=NCB)
    ov = out.rearrange("p (b ci cb t) -> p b ci cb t", b=B_loc, ci=NCI, cb=NCB)

    with ExitStack() as ctx:
        tc = ctx.enter_context(tile.TileContext(nc))
        const = ctx.enter_context(tc.tile_pool(name="const", bufs=1))
        xpool = ctx.enter_context(
            tc.tile_pool(name="xp", bufs=2 * (PREF + 1) + 1))
        opool = ctx.enter_context(tc.tile_pool(name="op", bufs=4))
        gpool = ctx.enter_context(tc.tile_pool(name="gp", bufs=4))
        upool = ctx.enter_context(tc.tile_pool(name="up", bufs=3))
        hpool = ctx.enter_context(tc.tile_pool(name="hp", bufs=3))
        cpool = ctx.enter_context(tc.tile_pool(name="cp", bufs=2))
        php = ctx.enter_context(tc.tile_pool(name="php", bufs=2, space="PSUM"))
        pgp = ctx.enter_context(tc.tile_pool(name="pgp", bufs=3, space="PSUM"))

        # Consts ride the Scalar HWDGE ring so the Sync ring starts on x
        # immediately.
        w1_t = const.tile([P, NCB * CSQ], F16, tag="w1e")
        nc.scalar.dma_start(w1_t[:], w1e[:])
        w2_t = const.tile([M2, C_], F16, tag="w2d")
        nc.scalar.dma_start(w2_t[:], w2d[:])
        b1_t = const.tile([M2, 1], F32, tag="b1d")
        nc.scalar.dma_start(b1_t[:], b1d[:])
        b2_t = const.tile([P, NCB], F32, tag="b2e")
        nc.scalar.dma_start(b2_t[:], b2e[:])
        dconst = const.tile([M2, TS], F32, tag="dconst")
        nc.vector.memset(dconst[:], d)

        xts = {}

        def emit_loads(ci):
            for b in range(B_loc):
                xt = xpool.tile([P, NCB * Tc], F16, tag="x", name=f"x{b}_{ci}")
                xw3 = xt[:].rearrange("p (cb t) -> p cb t", cb=NCB)
                nc.sync.dma_start(xw3[:, :, :], xv[:, b, ci, :, :])
                xts[(b, ci)] = xt

        for ci in range(min(PREF, NTH)):
            emit_loads(ci)

        ph_pre = {}

        def phase1(ci):
            # mm1 for chunk ci, both batches stacked into one PSUM tile
            # (b0 -> rows 0-31, b1 -> rows 32-63 via PE tile placement).
            # Emitted one chunk ahead so the PE never sits behind a
            # relu-blocked mm2 while independent mm1 work exists.  The
            # two batches' accumulation chains are emission-interleaved
            # so PE col-tiles (0,0) and (0,32) co-execute.
            xws_ = [xts[(b, ci)][:].rearrange("p (cb t) -> p cb t", cb=NCB)
                    for b in range(B_loc)]
            phs = []
            for ts in range(Tc // TS):
                ph = php.tile([M2, TS], F32, tag="ph")
                for cb in range(NCB):
                    for b in range(B_loc):
                        nc.tensor.matmul(
                            ph[b * CSQ:(b + 1) * CSQ, :],
                            w1_t[:, cb * CSQ:(cb + 1) * CSQ],
                            xws_[b][:, cb, ts * TS:(ts + 1) * TS],
                            start=(cb == 0), stop=(cb == NCB - 1))
                phs.append(ph)
            ph_pre[ci] = phs

        phase1(0)
        carry = None
        for th in range(NTH):
            if th + PREF < NTH:
                emit_loads(th + PREF)
            if th + 1 < NTH:
                phase1(th + 1)
            phs = ph_pre.pop(th)
            # Phase 2: scan per sub-tile + one fused relu per chunk,
            # covering BOTH batches (stacked rows).
            ut = upool.tile([M2, Tc], F32, tag="u")
            for ts in range(Tc // TS):
                if th == 0 and ts == 0:
                    # u_0 = cw * p_0 makes y[0] = x[0] exact.
                    init = cpool.tile([M2, 1], F32, tag="c")
                    nc.vector.tensor_scalar_mul(
                        init[:], phs[ts][:, 0:1], float(cw))
                    init_ap = init[:]
                else:
                    init_ap = carry
                nc.vector.tensor_tensor_scan(
                    ut[:, ts * TS:(ts + 1) * TS], dconst[:],
                    phs[ts][:], init_ap,
                    mybir.AluOpType.mult, mybir.AluOpType.add)
                carry = ut[:, (ts + 1) * TS - 1:(ts + 1) * TS]
            # Fused (u + b1) -> max(., 0) on the DVE keeps ACT free for
            # sigmoids.
            ht = hpool.tile([M2, Tc], F16, tag="h")
            nc.vector.tensor_scalar(
                ht[:], ut[:], b1_t[:], 0.0,
                mybir.AluOpType.add, mybir.AluOpType.max)
            # Phase 3: mm2 + sigmoid per (b, cb); both sub-tiles of the
            # chunk land in one PSUM tile -> one sigmoid each, b2 riding
            # the ACTIVATE bias.  b0/b1 interleaved (PE row-tiles T0/T4).
            gts = [gpool.tile([P, NCB * Tc], F16, tag="g", name=f"g{b}")
                   for b in range(B_loc)]
            gws = [g[:].rearrange("p (cb t) -> p cb t", cb=NCB) for g in gts]
            for cb in range(NCB):
                pgs = [pgp.tile([P, Tc], F32, tag="pg", name=f"pg{b}")
                       for b in range(B_loc)]
                for ts in range(Tc // TS):
                    for b in range(B_loc):
                        nc.tensor.matmul(
                            pgs[b][:, ts * TS:(ts + 1) * TS],
                            w2_t[b * CSQ:(b + 1) * CSQ, cb * P:(cb + 1) * P],
                            ht[b * CSQ:(b + 1) * CSQ, ts * TS:(ts + 1) * TS],
                            start=True, stop=True)
                for b in range(B_loc):
                    nc.scalar.activation(
                        gws[b][:, cb, :], pgs[b][:],
                        mybir.ActivationFunctionType.Sigmoid,
                        bias=b2_t[:, cb:cb + 1])
            # Phase 4: gate multiply into a fresh fp16 tile (all-16-bit,
            # packed DVE rate), one piece per cb-pair so each store can
            # stream as soon as its half is gated.  Stores stay on the
            # Sync ring with the loads.
            for b in range(B_loc):
                ot = opool.tile([P, NCB * Tc], F16, tag="o", name=f"o{b}")
                ow = ot[:].rearrange("p (cb t) -> p cb t", cb=NCB)
                xw = xts.pop((b, th))[:].rearrange(
                    "p (cb t) -> p cb t", cb=NCB)
                for cbp in range(0, NCB, 2):
                    nc.vector.tensor_mul(
                        ow[:, cbp:cbp + 2, :],
                        xw[:, cbp:cbp + 2, :],
                        gws[b][:, cbp:cbp + 2, :])
                    nc.sync.dma_start(
                        ov[:, b, th, cbp:cbp + 2, :],
                        ow[:, cbp:cbp + 2, :])
    nc.compile()
    return nc


def make_in_maps(x, w1, b1, w2, b2, cw, n_cores=N_CORES):
    """Host-side shard + weight prep. Returns per-core input maps."""
    a = 1.0 / cw
    C_ = w2.shape[0]
    b_loc = x.shape[0] // n_cores

    w1sT = (np.asarray(w1) * a).T.astype(np.float32)      # [C, CSQ]
    w1e = np.empty((P, NCB * CSQ), dtype=np.float16)
    for cb in range(NCB):
        w1e[:, cb * CSQ:(cb + 1) * CSQ] = w1sT[cb * P:(cb + 1) * P, :]

    w2d = np.empty((M2, C_), dtype=np.float16)
    for b in range(b_loc):
        w2d[b * CSQ:(b + 1) * CSQ, :] = np.asarray(w2).T

    b1d = np.empty((M2, 1), dtype=np.float32)
    for b in range(b_loc):
        b1d[b * CSQ:(b + 1) * CSQ, 0] = np.asarray(b1)

    b2e = np.asarray(b2).astype(np.float32).reshape(NCB, P).T.copy()

    # [B, C, T] -> per-core [P, b, ci, cb, t] fp16 (see build_nc).
    x16 = np.asarray(x).astype(np.float16)
    x16 = x16.reshape(n_cores, b_loc, NCB, P, NCI, Tc)
    x16 = np.ascontiguousarray(x16.transpose(0, 3, 1, 4, 2, 5))
    x16 = x16.reshape(n_cores, P, b_loc * NCI * NCB * Tc)

    return [
        {"x": x16[i], "w1e": w1e, "w2d": w2d, "b1d": b1d, "b2e": b2e}
        for i in range(n_cores)
    ]


def unshard_out(results, n_cores=N_CORES, b_loc=B_LOC):
    """Per-core [P, b*ci*cb*t] fp16 -> full [B, C, T] fp32."""
    o = np.stack([r["out"] for r in results], axis=0)
    o = o.reshape(n_cores, P, b_loc, NCI, NCB, Tc)
    o = o.transpose(0, 2, 4, 1, 3, 5)          # [core, b, cb, p, ci, t]
    return np.ascontiguousarray(o).reshape(B, C, T).astype(np.float32)


_NC_CACHE = {}


def kernel(x, w1, b1, w2, b2, context_window):
    cw = int(context_window)
    x = np.asarray(x)
    key = (cw, x.shape)
    if key not in _NC_CACHE:
        _NC_CACHE[key] = build_nc(x.shape[0] // N_CORES, cw)
    nc = _NC_CACHE[key]
    in_maps = make_in_maps(
        np.asarray(x), np.asarray(w1), np.asarray(b1),
        np.asarray(w2), np.asarray(b2), cw)
    res = run_bass_kernel_spmd(nc, in_maps, core_ids=list(range(N_CORES)))
    return unshard_out(res.results)


# revision 35
# speedup vs baseline: 1.2042x; 1.0516x over previous
"""Causal squeeze-excite 1d on 8 TRN2 NeuronCores.

Reference computation (per batch b):
    y = causal_ema(x)                      # y[t] = (1-a) y[t-1] + a x[t], y[0] = x[0]
    h = relu(w1 @ y[:, t] + b1)            # (32,)  per time step
    g = sigmoid(w2 @ h + b2)               # (512,) per time step
    out[:, t] = x[:, t] * g
Sharding: data-parallel over batch; core i gets x[2i:2i+2].

Structure (fp16 IO):
  - x/out/weights travel as fp16: halves HBM traffic (the kernel is
    DMA-bound at ~358 GB/s/core); fp16's 2^-11 rounding is far inside
    tolerance.  Host lays DRAM out as [128p, b, chunk, cb, t] so every
    load is 128 descriptors x 8 KB contiguous.
  - EMA commutes with the channel projection: w1 @ ema(x) == ema((a*w1) @ x),
    so the DVE scan runs on a 32-row projected sequence, not [512, T].
  - Both batches stack in PSUM partitions (b0 rows 0-31, b1 rows 32-63 via
    PE tile placement), so ONE scan / ONE relu instruction covers both
    batches.  b0/b1 matmul chains are emission-interleaved so the two PE
    tiles co-execute (~2x PE throughput, robust to HAM throttling).
  - b1 rides the DVE relu (fused add+max); b2 rides the sigmoid
    ACTIVATE's per-partition bias.  ACT runs sigmoids only -- it is the
    busiest compute engine, so everything else stays off it; uniform
    1024-col chunks minimize the (N+352)-cycle ACTIVATE overhead.
"""

import numpy as np
from contextlib import ExitStack

import concourse.bass as bass
import concourse.bacc as bacc
import concourse.tile as tile
import concourse.mybir as mybir
from concourse.bass_utils import run_bass_kernel_spmd

F32 = mybir.dt.float32
F16 = mybir.dt.float16

N_CORES = 8
B, C, T = 16, 512, 4096
CSQ = 32          # squeeze dim
P = 128           # SBUF partitions
NCB = C // P      # channel blocks (4)
B_LOC = B // N_CORES          # batches per core (2)
M2 = B_LOC * CSQ  # stacked mm1 output rows (64)
Tc = 1024         # time chunk
NCI = T // Tc     # DRAM chunk blocks (4)
TS = 512          # matmul / scan sub-tile (one PSUM bank)
PREF = 2          # load prefetch distance, in chunks
CHUNKS = [(0, 1024), (1024, 1024), (2048, 1024), (3072, 1024)]
NTH = len(CHUNKS)


def build_nc(B_loc, cw, C_=C, T_=T):
    assert B_loc == B_LOC
    d = 1.0 - 1.0 / cw

    nc = bacc.Bacc(trn_type="TRN2")
    # x/out DRAM layout: [p, b, ci, cb, t] (fp16).  One load per (b, ci)
    # is 128 x 8KB contiguous; one store per (b, ci, cb-pair) is 128 x 4KB.
    xin = nc.declare_dram_parameter("x", [P, B_loc * NCI * NCB * Tc], F16,
                                    isOutput=False)
    w1e = nc.declare_dram_parameter("w1e", [P, NCB * CSQ], F16, isOutput=False)
    w2d = nc.declare_dram_parameter("w2d", [M2, C_], F16, isOutput=False)
    b1d = nc.declare_dram_parameter("b1d", [M2, 1], F32, isOutput=False)
    b2e = nc.declare_dram_parameter("b2e", [P, NCB], F32, isOutput=False)
    out = nc.declare_dram_parameter("out", [P, B_loc * NCI * NCB * Tc], F16,
                                    isOutput=True)

    xv = xin.rearrange("p (b ci cb t) -> p b ci cb t", b=B_loc, ci=NCI, cb=NCB)
    ov = out.rearrange("p (b ci cb t) -> p b ci cb t", b=B_loc, ci=NCI, cb=NCB)

    with ExitStack() as ctx:
        tc = ctx.enter_context(tile.TileContext(nc))
        const = ctx.enter_context(tc.tile_pool(name="const", bufs=1))
        xpool = ctx.enter_context(
            tc.tile_pool(name="xp", bufs=2 * (PREF + 1) + 1))
        opool = ctx.enter_context(tc.tile_pool(name="op", bufs=4))
        gpool = ctx.enter_context(tc.tile_pool(name="gp", bufs=4))
        upool = ctx.enter_context(tc.tile_pool(name="up", bufs=3))
        hpool = ctx.enter_context(tc.tile_pool(name="hp", bufs=3))
        cpool = ctx.enter_context(tc.tile_pool(name="cp", bufs=2))
        php = ctx.enter_context(tc.tile_pool(name="php", bufs=2, space="PSUM"))
        pgp = ctx.enter_context(tc.tile_pool(name="pgp", bufs=3, space="PSUM"))

        # Consts ride the Scalar HWDGE ring so the Sync ring starts on x
        # immediately.
        w1_t = const.tile([P, NCB * CSQ], F16, tag="w1e")
        nc.scalar.dma_start(w1_t[:], w1e[:])
        w2_t = const.tile([M2, C_], F16, tag="w2d")
        nc.scalar.dma_start(w2_t[:], w2d[:])
        b1_t = const.tile([M2, 1], F32, tag="b1d")
        nc.scalar.dma_start(b1_t[:], b1d[:])
        b2_t = const.tile([P, NCB], F32, tag="b2e")
        nc.scalar.dma_start(b2_t[:], b2e[:])
        dconst = const.tile([M2, TS], F32, tag="dconst")
        nc.vector.memset(dconst[:], d)

        xts = {}

        def emit_loads(ci):
            for b in range(B_loc):
                xt = xpool.tile([P, NCB * Tc], F16, tag="x", name=f"x{b}_{ci}")
                xw3 = xt[:].rearrange("p (cb t) -> p cb t", cb=NCB)
                nc.sync.dma_start(xw3[:, :, :], xv[:, b, ci, :, :])
                xts[(b, ci)] = xt

        for ci in range(min(PREF, NTH)):
            emit_loads(ci)

        # Warm the PE while the first x chunk is still in flight: HAM
        # grants the full K=8/8 clock only after ~3.4us of sustained MM
        # activity, so ~24 dummy matmuls on the (tiny, already-loaded)
        # w1 tile let the first REAL mm1 run at full rate instead of
        # paying the cold-ramp on the critical spine.
        scratch = php.tile([M2, TS], F32, tag="ph", name="warm")
        for _ in range(24):
            nc.tensor.matmul(scratch[0:CSQ, 0:NCB * CSQ],
                             w1_t[:, 0:CSQ], w1_t[:],
                             start=True, stop=True)

        ph_pre = {}

        def phase1(ci):
            # mm1 for chunk ci, both batches stacked into one PSUM tile
            # (b0 -> rows 0-31, b1 -> rows 32-63 via PE tile placement).
            # Emitted one chunk ahead so the PE never sits behind a
            # relu-blocked mm2 while independent mm1 work exists.  The
            # two batches' accumulation chains are emission-interleaved
            # so PE col-tiles (0,0) and (0,32) co-execute.
            xws_ = [xts[(b, ci)][:].rearrange("p (cb t) -> p cb t", cb=NCB)
                    for b in range(B_loc)]
            phs = []
            for ts in range(Tc // TS):
                ph = php.tile([M2, TS], F32, tag="ph")
                for cb in range(NCB):
                    for b in range(B_loc):
                        nc.tensor.matmul(
                            ph[b * CSQ:(b + 1) * CSQ, :],
                            w1_t[:, cb * CSQ:(cb + 1) * CSQ],
                            xws_[b][:, cb, ts * TS:(ts + 1) * TS],
                            start=(cb == 0), stop=(cb == NCB - 1))
                phs.append(ph)
            ph_pre[ci] = phs

        phase1(0)
        carry = None
        for th in range(NTH):
            if th + PREF < NTH:
                emit_loads(th + PREF)
            if th + 1 < NTH:
                phase1(th + 1)
            phs = ph_pre.pop(th)
            # Phase 2: scan per sub-tile + one fused relu per chunk,
            # covering BOTH batches (stacked rows).
            ut = upool.tile([M2, Tc], F32, tag="u")
            for ts in range(Tc // TS):
                if th == 0 and ts == 0:
                    # u_0 = cw * p_0 makes y[0] = x[0] exact.
                    init = cpool.tile([M2, 1], F32, tag="c")
                    nc.vector.tensor_scalar_mul(
                        init[:], phs[ts][:, 0:1], float(cw))
                    init_ap = init[:]
                else:
                    init_ap = carry
                nc.vector.tensor_tensor_scan(
                    ut[:, ts * TS:(ts + 1) * TS], dconst[:],
                    phs[ts][:], init_ap,
                    mybir.AluOpType.mult, mybir.AluOpType.add)
                carry = ut[:, (ts + 1) * TS - 1:(ts + 1) * TS]
            # Fused (u + b1) -> max(., 0) on the DVE keeps ACT free for
            # sigmoids.
            ht = hpool.tile([M2, Tc], F16, tag="h")
            nc.vector.tensor_scalar(
                ht[:], ut[:], b1_t[:], 0.0,
                mybir.AluOpType.add, mybir.AluOpType.max)
            # Phase 3: mm2 + sigmoid per (b, cb); both sub-tiles of the
            # chunk land in one PSUM tile -> one sigmoid each, b2 riding
            # the ACTIVATE bias.  b0/b1 interleaved (PE row-tiles T0/T4).
            gts = [gpool.tile([P, NCB * Tc], F16, tag="g", name=f"g{b}")
                   for b in range(B_loc)]
            gws = [g[:].rearrange("p (cb t) -> p cb t", cb=NCB) for g in gts]
            for cb in range(NCB):
                pgs = [pgp.tile([P, Tc], F32, tag="pg", name=f"pg{b}")
                       for b in range(B_loc)]
                for ts in range(Tc // TS):
                    for b in range(B_loc):
                        nc.tensor.matmul(
                            pgs[b][:, ts * TS:(ts + 1) * TS],
                            w2_t[b * CSQ:(b + 1) * CSQ, cb * P:(cb + 1) * P],
                            ht[b * CSQ:(b + 1) * CSQ, ts * TS:(ts + 1) * TS],
                            start=True, stop=True)
                for b in range(B_loc):
                    nc.scalar.activation(
                        gws[b][:, cb, :], pgs[b][:],
                        mybir.ActivationFunctionType.Sigmoid,
                        bias=b2_t[:, cb:cb + 1])
            # Phase 4: gate multiply into a fresh fp16 tile (all-16-bit,
            # packed DVE rate), one piece per cb-pair so each store can
            # stream as soon as its half is gated.  Stores stay on the
            # Sync ring with the loads.
            for b in range(B_loc):
                ot = opool.tile([P, NCB * Tc], F16, tag="o", name=f"o{b}")
                ow = ot[:].rearrange("p (cb t) -> p cb t", cb=NCB)
                xw = xts.pop((b, th))[:].rearrange(
                    "p (cb t) -> p cb t", cb=NCB)
                for cbp in range(0, NCB, 2):
                    nc.vector.tensor_mul(
                        ow[:, cbp:cbp + 2, :],
                        xw[:, cbp:cbp + 2, :],
                        gws[b][:, cbp:cbp + 2, :])
                    nc.sync.dma_start(
                        ov[:, b, th, cbp:cbp + 2, :],
                        ow[:, cbp:cbp + 2, :])
    nc.compile()
    return nc


def make_in_maps(x, w1, b1, w2, b2, cw, n_cores=N_CORES):
    """Host-side shard + weight prep. Returns per-core input maps."""
    a = 1.0 / cw
    C_ = w2.shape[0]
    b_loc = x.shape[0] // n_cores

    w1sT = (np.asarray(w1) * a).T.astype(np.float32)      # [C, CSQ]
    w1e = np.empty((P, NCB * CSQ), dtype=np.float16)
    for cb in range(NCB):
        w1e[:, cb * CSQ:(cb + 1) * CSQ] = w1sT[cb * P:(cb + 1) * P, :]

    w2d = np.empty((M2, C_), dtype=np.float16)
    for b in range(b_loc):
        w2d[b * CSQ:(b + 1) * CSQ, :] = np.asarray(w2).T

    b1d = np.empty((M2, 1), dtype=np.float32)
    for b in range(b_loc):
        b1d[b * CSQ:(b + 1) * CSQ, 0] = np.asarray(b1)

    b2e = np.asarray(b2).astype(np.float32).reshape(NCB, P).T.copy()

    # [B, C, T] -> per-core [P, b, ci, cb, t] fp16 (see build_nc).
    x16 = np.asarray(x).astype(np.float16)
    x16 = x16.reshape(n_cores, b_loc, NCB, P, NCI, Tc)
    x16 = np.ascontiguousarray(x16.transpose(0, 3, 1, 4, 2, 5))
    x16 = x16.reshape(n_cores, P, b_loc * NCI * NCB * Tc)

    return [
        {"x": x16[i], "w1e": w1e, "w2d": w2d, "b1d": b1d, "b2e": b2e}
        for i in range(n_cores)
    ]


def unshard_out(results, n_cores=N_CORES, b_loc=B_LOC):
    """Per-core [P, b*ci*cb*t] fp16 -> full [B, C, T] fp32."""
    o = np.stack([r["out"] for r in results], axis=0)
    o = o.reshape(n_cores, P, b_loc, NCI, NCB, Tc)
    o = o.transpose(0, 2, 4, 1, 3, 5)          # [core, b, cb, p, ci, t]
    return np.ascontiguousarray(o).reshape(B, C, T).astype(np.float32)


_NC_CACHE = {}


def kernel(x, w1, b1, w2, b2, context_window):
    cw = int(context_window)
    x = np.asarray(x)
    key = (cw, x.shape)
    if key not in _NC_CACHE:
        _NC_CACHE[key] = build_nc(x.shape[0] // N_CORES, cw)
    nc = _NC_CACHE[key]
    in_maps = make_in_maps(
        np.asarray(x), np.asarray(w1), np.asarray(b1),
        np.asarray(w2), np.asarray(b2), cw)
    res = run_bass_kernel_spmd(nc, in_maps, core_ids=list(range(N_CORES)))
    return unshard_out(res.results)
